# revision 1
# baseline (speedup 1.0000x reference)
"""Trainium2 Bass kernel for nn_Attention_75299366633572.

Math (reference):
    scale[s] = temporal-PE flattened, s in [0, 1024)
    xs[n,s,:] = x[n,s,:] * scale[s]
    h = xs @ W.T + b                       # [N, S, 384]
    q,k,v = interleaved split of h         # each [N, S*128] via h[...,0::3] etc.
    scores = q @ k.T / sqrt(128)           # [128, 128]  (attention over batch!)
    out = softmax(scores) @ v              # [128, 131072]

Key algebraic restructure (per position s, with Wq' = Wq/sqrt(128)):
    scores[n,m] = sum_s xs_s[n,:] @ A @ xs_s[m,:].T  + (w . xs_s[m,:]) + rowconst
        A = Wq'.T @ Wk   [128,128],   w = Wk.T @ bq'  (bias term varying over m)
    row-constant terms (q_n.bk etc.) are softmax-invariant -> dropped.
    v bias: softmax rows sum to 1 -> out[n, (s,g)] += bv[g] added at the end.

Sharding: S (sequence) dim split across 8 cores (128 positions each).
Each core computes a partial [128,128] score matrix -> tiny AllGather +
on-chip sum -> replicated softmax -> each core emits its 16384 output cols.

Host prep (layout only): scale*x fused into a transpose to xs^T per core
([d, (s,n)]), pre-rounded to fp32r (TF32-like) so matmuls take the
single-pass fp32r path; small derived matrices A, w, Wv^T, bv.

Per-core device pipeline (all matmuls fp32r unless noted):
  1. DMA XT = xs^T slice [128, 16384].
  2. per 512-col chunk: YT = A^T @ XT_chunk (+w bias fused in PSUM->SBUF copy)
     then 4 accumulating score matmuls  scores += YT_s^T @ XT_s.
  3. AllGather partial scores (64 KiB) + 3 tree adds -- overlapped with:
  4. V_s = xs_s @ Wv^T  (per s; PE keeps running through the collective)
  5. softmax (fp32, replicated), attnT = attn^T
  6. out_chunk = attnT^T @ V_chunk + bv -> DMA out
"""

import math

import numpy as np

import concourse.bass as bass
import concourse.mybir as mybir
import concourse.tile as tile
from concourse import bacc
from concourse.bass_utils import run_bass_kernel_spmd
from concourse.masks import make_identity

NCORES = 8
N = 128            # batch rows (attention is over this axis)
S = 1024           # sequence positions
D = 128            # feature dim
S_LOC = S // NCORES       # 128 positions per core
COLS = S_LOC * D          # 16384 free columns per core
F32 = mybir.dt.float32
F32R = mybir.dt.float32r
F16 = mybir.dt.float16

_CACHE = {}


def _temporal_scale():
    """pe.flatten() from the reference's _temporal_pe, float32."""
    i = np.arange(32, dtype=np.float32)[:, None]
    j = np.arange(16, dtype=np.float32)[None, :]
    arg = (np.float32(1.0) * np.float32(np.pi) * i
           / np.power(np.float32(1000.0), (np.float32(2.0) * j / np.float32(128.0))))
    pe = np.stack([np.sin(arg), np.cos(arg)], axis=-1).reshape(32, 32)
    return pe.reshape(-1).astype(np.float32)   # [1024]


def _round_f32r(a):
    """Round-to-nearest-even to 11 explicit mantissa bits (= the precision
    TRN2's fp32r keeps, verified on hardware)."""
    u = np.ascontiguousarray(a, dtype=np.float32).view(np.uint32)
    even = (u >> np.uint32(12)) & np.uint32(1)
    rounded = (u + np.uint32(0x07FF) + even) & np.uint32(0xFFFFF000)
    return rounded.view(np.float32)


def _emit(nc, tc, xt_d, A_d, w_d, WvT_d, out_d):
    AX = mybir.AxisListType
    AF = mybir.ActivationFunctionType

    with (
        tc.tile_pool(name="consts", bufs=1) as consts,
        tc.tile_pool(name="xt", bufs=1) as xtp,
        tc.tile_pool(name="vbuf", bufs=1) as vp,
        tc.tile_pool(name="small", bufs=1) as small,
        tc.tile_pool(name="dram", bufs=1, space="DRAM") as dram,
    ):
        ident = consts.tile([128, 128], F32)
        make_identity(nc, ident[:])
        A_sb = consts.tile([D, D], F32R)
        nc.sync.dma_start(A_sb[:], A_d[:])
        WvT_sb = consts.tile([D, D], F32)
        w_sb = consts.tile([D, 1], F32)
        nc.sync.dma_start(w_sb[:], w_d[:])

        XT = xtp.tile([128, COLS], F32R)     # xs^T, [d, (s,n)], pre-rounded
        V = vp.tile([128, COLS], F16)        # v rows, [m, (s,g)]

        sc_part = small.tile([128, 128], F32, tag="scpart")
        ag_sb = small.tile([128, 8 * 128], F32, tag="ag")
        t512 = small.tile([128, 512], F32, tag="t512")
        t256 = small.tile([128, 256], F32, tag="t256")
        sc_full = small.tile([128, 128], F32, tag="scfull")
        ex = small.tile([128, 128], F32, tag="ex")
        attn = small.tile([128, 128], F32, tag="attn")
        attnT = small.tile([128, 128], F16, tag="attnT")
        mx = small.tile([128, 1], F32, tag="mx")
        sume = small.tile([128, 1], F32, tag="sume")
        rinv = small.tile([128, 1], F32, tag="rinv")

        in_bounce = dram.tile([128, 128], F32)
        ag_bounce = dram.tile([8 * 128, 128], F32)

        # XT input: one HWDGE queue (each dma_start already fans across all
        # 16 DMA engines; extra queues only add contention), in ascending
        # column order so the first Y matmul starts after ~256 KiB.
        bounds = [0, 128, 256, 512, 1024] + [1024 + 1024 * i for i in range(1, 16)]
        for i, (lo, hi) in enumerate(zip(bounds[:-1], bounds[1:])):
            nc.sync.dma_start(XT[:, lo:hi], xt_d[:, lo:hi])
            if i == 3:
                nc.sync.dma_start(WvT_sb[:], WvT_d[:])

        # Warm-up: the PE's HAM clock gate starts at 1.2 GHz and only
        # reaches 2.4 GHz after ~3.4us of sustained activity. Burn dummy
        # transposes inside the first-chunk DMA wait so phase 1 starts warm.
        with tc.tile_pool(name="ps_wu", bufs=1, space="PSUM") as ps_wu:
            wps = ps_wu.tile([128, 128], F32)
            for _ in range(30):
                nc.tensor.transpose(wps[:], ident[:], ident[:])

        # ---- Phase 1: Y = A^T @ XT (+w) and partial scores ----
        with (
            tc.tile_pool(name="yt", bufs=4) as ytp,
            tc.tile_pool(name="ps_y", bufs=3, space="PSUM") as ps_y,
            tc.tile_pool(name="ps_sc", bufs=1, space="PSUM") as ps_sc,
        ):
            sc_ps = ps_sc.tile([128, 128], F32)
            for c in range(COLS // 512):          # 32 chunks of 512 cols (4 s)
                yps = ps_y.tile([128, 512], F32, tag="y")
                nc.tensor.matmul(yps[:], A_sb[:], XT[:, c * 512:(c + 1) * 512],
                                 start=True, stop=True)
                yt = ytp.tile([128, 512], F32R, tag="yt")
                nc.vector.tensor_scalar_add(yt[:], yps[:], w_sb[:, 0:1])
                for k in range(4):
                    s = 4 * c + k
                    nc.tensor.matmul(sc_ps[:], yt[:, k * 128:(k + 1) * 128],
                                     XT[:, s * 128:(s + 1) * 128],
                                     start=(s == 0), stop=(s == S_LOC - 1))
            sc_done = nc.vector.tensor_copy(sc_part[:], sc_ps[:])

        # ---- AllGather partial scores; sum the 8 slices on DVE ----
        nc.sync.dma_start(in_bounce[:], sc_part[:])
        nc.gpsimd.collective_compute(
            "AllGather", mybir.AluOpType.bypass,
            replica_groups=[list(range(NCORES))],
            ins=[in_bounce[:].opt()], outs=[ag_bounce[:].opt()],
        )
        nc.sync.dma_start(
            ag_sb[:, 0:512].rearrange("p (r j) -> p r j", r=4),
            ag_bounce[0:512, :].rearrange("(r p) j -> p r j", p=128))
        rb2 = nc.scalar.dma_start(
            ag_sb[:, 512:1024].rearrange("p (r j) -> p r j", r=4),
            ag_bounce[512:1024, :].rearrange("(r p) j -> p r j", p=128))

        # ---- Phase 2: V projection (PE stays busy through the collective).
        # Explicitly held AFTER the score matmuls so this ~35us of PE work
        # hides the collective's ~27us latency instead of being front-run
        # into phase 1 by the scheduler.
        v_copy_dve = v_copy_act = None
        with tc.tile_pool(name="ps_v", bufs=6, space="PSUM") as ps_v:
            for s in range(S_LOC):
                vps = ps_v.tile([128, 128], F32, tag="v")
                vm = nc.tensor.matmul(vps[:],
                                      XT[:, s * 128:(s + 1) * 128].bitcast(F32),
                                      WvT_sb[:], start=True, stop=True)
                tile.add_dep_helper(vm.ins, sc_done.ins, sync=True,
                                    reason="run V after scores to hide AG")
                dst = V[:, s * 128:(s + 1) * 128]
                if s % 2 == 0:
                    v_copy_dve = nc.vector.tensor_copy(dst, vps[:])
                else:
                    v_copy_act = nc.scalar.copy(dst, vps[:])

        # ---- sum AG slices + softmax + attn transpose ----
        # Keep the post-collective dependency chain SHORT (each cross-engine
        # hop costs ~0.5-5us in sem latency here): the 3 tree adds run on
        # GPSIMD (idle; immune to the in-order DVE/ACT V-copy streams), exp
        # uses a CONSTANT -40 bias instead of a row-max (softmax-invariant;
        # logits are < ~70 for this problem so no overflow), the 1/rowsum is
        # folded into the per-partition scale of the output copies, and the
        # transpose runs on the raw exp directly.
        nc.gpsimd.tensor_add(t512[:], ag_sb[:, 0:512], ag_sb[:, 512:1024])
        nc.gpsimd.tensor_add(t256[:], t512[:, 0:256], t512[:, 256:512])
        nc.gpsimd.tensor_add(sc_full[:], t256[:, 0:128], t256[:, 128:256])
        rmax = nc.vector.reduce_max(out=mx[:], in_=sc_full[:], axis=AX.X,
                                    negate=True)
        tile.add_dep_helper(rmax.ins, v_copy_dve.ins, sync=True,
                            reason="row-max after last DVE V copy")
        expi = nc.scalar.activation(ex[:], sc_full[:], AF.Exp,
                                    bias=mx[:, 0:1], scale=1.0,
                                    accum_out=sume[:, 0:1])
        # ACT/DVE execute their streams in order: if exp (or the attnT copy)
        # were scheduled before the tail of the V copies and the collective
        # ran long, the V pipeline would stall behind it. Pin them after.
        tile.add_dep_helper(expi.ins, v_copy_act.ins, sync=True,
                            reason="exp after last ACT V copy")
        tile.add_dep_helper(rb2.ins, v_copy_act.ins, sync=True,
                            reason="AG readback half 2 after last ACT V copy")
        nc.vector.reciprocal(rinv[:], sume[:])
        with tc.tile_pool(name="ps_at", bufs=1, space="PSUM") as ps_at:
            atps = ps_at.tile([128, 128], F32)
            nc.tensor.transpose(atps[:], ex[:], ident[:])
            atc = nc.vector.tensor_copy(attnT[:], atps[:])
            tile.add_dep_helper(atc.ins, v_copy_dve.ins, sync=True,
                                reason="attnT copy after last DVE V copy")

        # ---- Phase 3: out = attnT^T @ V + bv ----
        with (
            tc.tile_pool(name="osb", bufs=8) as osbp,
            tc.tile_pool(name="ps_o", bufs=7, space="PSUM") as ps_o,
        ):
            for c in range(COLS // 512):
                ops = ps_o.tile([128, 512], F32, tag="o")
                nc.tensor.matmul(ops[:], attnT[:], V[:, c * 512:(c + 1) * 512],
                                 start=True, stop=True)
                osb = osbp.tile([128, 512], F16, tag="osb")
                nc.vector.tensor_scalar_mul(osb[:, 0:256], ops[:, 0:256],
                                            rinv[:, 0:1])
                nc.scalar.mul(osb[:, 256:512], ops[:, 256:512], rinv[:, 0:1])
                eng = [nc.sync, nc.scalar, nc.gpsimd][c % 3]
                eng.dma_start(out_d[:, c * 512:(c + 1) * 512], osb[:])


def _build():
    key = "v2"
    if key in _CACHE:
        return _CACHE[key]
    nc = bacc.Bacc("TRN2", target_bir_lowering=False, debug=False,
                   num_devices=NCORES)
    xt_d = nc.dram_tensor("xt", [128, COLS], F32R, kind="ExternalInput")
    A_d = nc.dram_tensor("A", [D, D], F32R, kind="ExternalInput")
    w_d = nc.dram_tensor("w", [D, 1], F32, kind="ExternalInput")
    WvT_d = nc.dram_tensor("WvT", [D, D], F32, kind="ExternalInput")
    out_d = nc.dram_tensor("out", [N, COLS], F16, kind="ExternalOutput")
    with tile.TileContext(nc) as tc:
        _emit(nc, tc, xt_d, A_d, w_d, WvT_d, out_d)
    nc.compile()
    _CACHE[key] = nc
    return nc


def prepare_inputs(x, W, b):
    """Host-side prep: shard + transpose x over S, build derived matrices."""
    x = np.asarray(x, dtype=np.float32)
    W = np.asarray(W, dtype=np.float32)
    b = np.asarray(b, dtype=np.float32)

    rs = math.sqrt(float(D))
    Wq = W[0::3, :].astype(np.float64) / rs
    Wk = W[1::3, :].astype(np.float64)
    Wv = W[2::3, :]
    bq = b[0::3].astype(np.float64) / rs
    bv = b[2::3]

    A = _round_f32r((Wq.T @ Wk).astype(np.float32))          # [128, 128]
    w = (Wk.T @ bq).astype(np.float32)[:, None]              # [128, 1]
    WvT = np.ascontiguousarray(Wv.T)                         # [128, 128]

    scale = _temporal_scale()                                # [1024]
    in_maps = []
    for c in range(NCORES):
        sl = slice(c * S_LOC, (c + 1) * S_LOC)
        xs_c = x[:, sl, :] * scale[sl][None, :, None]        # [n, s, d] f32
        xt_c = _round_f32r(
            np.ascontiguousarray(xs_c.transpose(2, 1, 0)).reshape(D, COLS))
        in_maps.append({
            "xt": xt_c, "A": A, "w": w, "WvT": WvT,
        })
    return in_maps, bv


def run(inputs, trace=False, **kw):
    nc = _build()
    in_maps, bv = prepare_inputs(inputs["x"], inputs["W"], inputs["b"])
    res = run_bass_kernel_spmd(nc, in_maps, core_ids=list(range(NCORES)),
                               trace=trace, **kw)
    out = np.concatenate(
        [res.results[c]["out"].astype(np.float32) for c in range(NCORES)], axis=1)
    out += np.tile(bv, S)[None, :]     # v-bias: attn rows sum to 1
    return out, res


def kernel(x, W, b):
    out, _ = run({"x": x, "W": W, "b": b})
    return out



# revision 2
# speedup vs baseline: 1.3837x; 1.3837x over previous
"""Trainium2 Bass kernel for nn_Attention_75299366633572.

Math (reference):
    scale[s] = temporal-PE flattened, s in [0, 1024)
    xs[n,s,:] = x[n,s,:] * scale[s]
    h = xs @ W.T + b                       # [N, S, 384]
    q,k,v = interleaved split of h         # each [N, S*128] via h[...,0::3] etc.
    scores = q @ k.T / sqrt(128)           # [128, 128]  (attention over batch!)
    out = softmax(scores) @ v              # [128, 131072]

Key algebraic restructure (per position s, with Wq' = Wq/sqrt(128)):
    scores[n,m] = sum_s xs_s[n,:] @ A @ xs_s[m,:].T + (w . xs_s[m,:]) + rowconst
        A = Wq'.T @ Wk   [128,128],   w = Wk.T @ bq'  (bias term varying over m)
    row-constant terms are softmax-invariant -> dropped.
    v bias: softmax rows sum to 1 -> out[n, (s,g)] += bv[g] added on host.

Sharding: S (sequence) split across 8 cores (128 positions each). Each core
computes a partial scores^T [m, n] -> 32 KiB fp16 AllGather -> transpose
readback -> on-chip sum -> replicated softmax -> each core emits its 16384
output cols.

v3 design notes (from trace analysis of the 153us baseline):
  * everything fp16 on device: halves input DMA (4 MiB/core), moving
    operands run 1 cy/col, and stationary loads get the 2x FWL path.
  * partials are scores^T so (a) V and score matmuls share the XT_s
    stationary operand and (b) the AllGather readback is a single
    HW DMA-transpose that lands scores in row-softmax orientation.
  * input streamed in 6 chunks alternating the two HWDGE rings (the
    baseline serialized 19 chunks on one ring at ~183 GB/s).
  * warmup uses real matmuls: PE-mode transposes don't count as activity
    for the HAM clock gate, so the baseline ran phase 1 at 1.2 GHz.
  * PSUM->SBUF copies widened to [128,512] (the [128,128] copies were
    ~290ns each, overhead-dominated).
  * 1/rowsum folded into attn before the transpose so phase-D copies are
    plain casts; collective payload cast to fp16 (32 KiB).
"""

import math

import numpy as np

import concourse.bass as bass
import concourse.mybir as mybir
import concourse.tile as tile
from concourse import bacc
from concourse.bass_utils import run_bass_kernel_spmd

NCORES = 8
N = 128            # batch rows (attention is over this axis)
S = 1024           # sequence positions
D = 128            # feature dim
S_LOC = S // NCORES       # 128 positions per core
COLS = S_LOC * D          # 16384 free columns per core
F32 = mybir.dt.float32
F16 = mybir.dt.float16

_CACHE = {}


def _temporal_scale():
    """pe.flatten() from the reference's _temporal_pe, float32."""
    i = np.arange(32, dtype=np.float32)[:, None]
    j = np.arange(16, dtype=np.float32)[None, :]
    arg = (np.float32(1.0) * np.float32(np.pi) * i
           / np.power(np.float32(1000.0), (np.float32(2.0) * j / np.float32(128.0))))
    pe = np.stack([np.sin(arg), np.cos(arg)], axis=-1).reshape(32, 32)
    return pe.reshape(-1).astype(np.float32)   # [1024]


def _emit(nc, tc, xt_d, A_d, w_d, WvT_d, id_d, out_d):
    AX = mybir.AxisListType
    AF = mybir.ActivationFunctionType

    with (
        tc.tile_pool(name="consts", bufs=1) as consts,
        tc.tile_pool(name="xt", bufs=1) as xtp,
        tc.tile_pool(name="yt", bufs=1) as ytp,
        tc.tile_pool(name="vbuf", bufs=1) as vp,
        tc.tile_pool(name="small", bufs=1) as small,
        tc.tile_pool(name="dram", bufs=1, space="DRAM") as dram,
    ):
        A_sb = consts.tile([D, D], F16)
        WvT_sb = consts.tile([D, D], F16)
        w_sb = consts.tile([D, 1], F32)
        ident = consts.tile([D, D], F16)

        XT = xtp.tile([128, COLS], F16)      # xs^T, [d, (s,n)]
        YT = ytp.tile([128, COLS], F16)      # y = A^T xs + w, [d', (s,n)]
        V = vp.tile([128, COLS], F16)        # v rows, [m, (s,g)]

        ag_sb = small.tile([128, 8 * 128], F16, tag="ag")
        t512 = small.tile([128, 512], F32, tag="t512")
        t256 = small.tile([128, 256], F32, tag="t256")
        sc_full = small.tile([128, 128], F32, tag="scfull")
        scT_sb = small.tile([128, 128], F16, tag="scT")
        ex = small.tile([128, 128], F16, tag="ex")
        exs = small.tile([128, 128], F16, tag="exs")
        attnT = small.tile([128, 128], F16, tag="attnT")
        mx = small.tile([128, 1], F32, tag="mx")
        sume = small.tile([128, 1], F32, tag="sume")
        rinv = small.tile([128, 1], F32, tag="rinv")

        in_bounce = dram.tile([128, 128], F16)
        ag_bounce = dram.tile([8 * 128, 128], F16)

        # Consts first (A leads: warmup + Y need it), then the XT stream
        # split across both HWDGE rings so each ring's next chunk loads
        # while the other ring's is consumed.
        nc.sync.dma_start(A_sb[:], A_d[:])
        nc.scalar.dma_start(ident[:], id_d[:])
        nc.scalar.dma_start(WvT_sb[:], WvT_d[:])
        nc.scalar.dma_start(w_sb[:], w_d[:])
        bounds = [0, 1024, 2048, 4096, 8192, 12288, 16384]
        for i, (lo, hi) in enumerate(zip(bounds[:-1], bounds[1:])):
            eng = nc.sync if i % 2 == 0 else nc.scalar
            eng.dma_start(XT[:, lo:hi], xt_d[:, lo:hi])

        # Warmup: REAL matmuls (transposes don't count as PE activity for
        # the HAM clock gate). Runs while the first XT chunk lands.
        with tc.tile_pool(name="ps_wu", bufs=1, space="PSUM") as ps_wu:
            wps = ps_wu.tile([128, 128], F32)
            for _ in range(12):
                nc.tensor.matmul(wps[:], A_sb[:], A_sb[:], start=True, stop=True)

        # ---- Phase A: Y = A^T @ XT (+w), scores^T accumulation ----
        # PE order (software-pipelined by one chunk so score matmuls never
        # wait on the cast of the chunk just produced):
        #   Y_0, Y_1, sc_0, Y_2, sc_1, ..., Y_31, sc_30, sc_31
        sc_mms = []

        def emit_sc_chunk(c):
            for k in range(4):
                s = 4 * c + k
                mm = nc.tensor.matmul(sc_ps[:],
                                      XT[:, s * 128:(s + 1) * 128],
                                      YT[:, s * 128:(s + 1) * 128],
                                      start=(s == 0), stop=(s == S_LOC - 1))
                sc_mms.append(mm)

        with (
            tc.tile_pool(name="ps_y", bufs=3, space="PSUM") as ps_y,
            tc.tile_pool(name="ps_sc", bufs=1, space="PSUM") as ps_sc,
        ):
            sc_ps = ps_sc.tile([128, 128], F32)
            for c in range(COLS // 512):          # 32 chunks of 512 cols (4 s)
                yps = ps_y.tile([128, 512], F32, tag="y")
                nc.tensor.matmul(yps[:], A_sb[:], XT[:, c * 512:(c + 1) * 512],
                                 start=True, stop=True)
                dst = YT[:, c * 512:(c + 1) * 512]
                if c % 2 == 0:
                    nc.vector.tensor_scalar_add(dst, yps[:], w_sb[:, 0:1])
                else:
                    nc.scalar.add(dst, yps[:], w_sb[:, 0:1])
                if c >= 1:
                    emit_sc_chunk(c - 1)
            emit_sc_chunk(31)
            sc_done = nc.vector.tensor_copy(scT_sb[:], sc_ps[:])

        # ---- AllGather of the fp16 partial scores^T ----
        nc.gpsimd.dma_start(in_bounce[:], scT_sb[:])
        nc.gpsimd.collective_compute(
            "AllGather", mybir.AluOpType.bypass,
            replica_groups=[list(range(NCORES))],
            ins=[in_bounce[:].opt()], outs=[ag_bounce[:].opt()],
        )

        # ---- V projection (hides the collective; PE keeps running) ----
        # Held after the score matmuls so it can't be front-run into
        # phase A by the scheduler.
        v_copy_dve = v_copy_act = None
        with tc.tile_pool(name="ps_v", bufs=3, space="PSUM") as ps_v:
            for g in range(S_LOC // 4):
                vps = ps_v.tile([128, 512], F32, tag="v")
                for k in range(4):
                    s = 4 * g + k
                    vm = nc.tensor.matmul(vps[:, k * 128:(k + 1) * 128],
                                          XT[:, s * 128:(s + 1) * 128],
                                          WvT_sb[:], start=True, stop=True)
                    if g == 0 and k == 0:
                        tile.add_dep_helper(vm.ins, sc_mms[-1].ins, sync=True,
                                            reason="run V after scores")
                dst = V[:, g * 512:(g + 1) * 512]
                if g % 2 == 0:
                    v_copy_dve = nc.vector.tensor_copy(dst, vps[:])
                else:
                    v_copy_act = nc.scalar.copy(dst, vps[:])

        # ---- readback (single HW transpose: lands scores row-major),
        #      sum, softmax, fold 1/rowsum, transpose to attnT ----
        rb = nc.sync.dma_start_transpose(ag_sb[:], ag_bounce[:])
        tile.add_dep_helper(rb.ins, sc_done.ins, sync=True,
                            reason="readback ordering")
        nc.vector.tensor_add(t512[:], ag_sb[:, 0:512], ag_sb[:, 512:1024])
        nc.vector.tensor_add(t256[:], t512[:, 0:256], t512[:, 256:512])
        a3 = nc.vector.tensor_add(sc_full[:], t256[:, 0:128], t256[:, 128:256])
        tile.add_dep_helper(a3.ins, v_copy_dve.ins, sync=True,
                            reason="adds after last DVE V copy")
        nc.vector.reduce_max(out=mx[:], in_=sc_full[:], axis=AX.X, negate=True)
        expi = nc.scalar.activation(ex[:], sc_full[:], AF.Exp,
                                    bias=mx[:, 0:1], scale=1.0,
                                    accum_out=sume[:, 0:1])
        tile.add_dep_helper(expi.ins, v_copy_act.ins, sync=True,
                            reason="exp after last ACT V copy")
        nc.vector.reciprocal(rinv[:], sume[:])
        nc.vector.tensor_scalar_mul(exs[:], ex[:], rinv[:, 0:1])
        with tc.tile_pool(name="ps_at", bufs=1, space="PSUM") as ps_at:
            atps = ps_at.tile([128, 128], F16)
            nc.tensor.transpose(atps[:], exs[:], ident[:])
            atc = nc.vector.tensor_copy(attnT[:], atps[:])
            tile.add_dep_helper(atc.ins, v_copy_dve.ins, sync=True,
                                reason="attnT copy after last DVE V copy")

        # ---- Phase D: out = attnT^T @ V (1/rowsum already folded in) ----
        with (
            tc.tile_pool(name="osb", bufs=8) as osbp,
            tc.tile_pool(name="ps_o", bufs=4, space="PSUM") as ps_o,
        ):
            for c in range(COLS // 512):
                ops = ps_o.tile([128, 512], F32, tag="o")
                nc.tensor.matmul(ops[:], attnT[:], V[:, c * 512:(c + 1) * 512],
                                 start=True, stop=True)
                osb = osbp.tile([128, 512], F16, tag="osb")
                if c % 2 == 0:
                    nc.vector.tensor_copy(osb[:], ops[:])
                else:
                    nc.scalar.copy(osb[:], ops[:])
                eng = [nc.sync, nc.scalar, nc.gpsimd][c % 3]
                eng.dma_start(out_d[:, c * 512:(c + 1) * 512], osb[:])


def _build():
    key = "v3"
    if key in _CACHE:
        return _CACHE[key]
    nc = bacc.Bacc("TRN2", target_bir_lowering=False, debug=False,
                   num_devices=NCORES)
    xt_d = nc.dram_tensor("xt", [128, COLS], F16, kind="ExternalInput")
    A_d = nc.dram_tensor("A", [D, D], F16, kind="ExternalInput")
    w_d = nc.dram_tensor("w", [D, 1], F32, kind="ExternalInput")
    WvT_d = nc.dram_tensor("WvT", [D, D], F16, kind="ExternalInput")
    id_d = nc.dram_tensor("ident", [D, D], F16, kind="ExternalInput")
    out_d = nc.dram_tensor("out", [N, COLS], F16, kind="ExternalOutput")
    with tile.TileContext(nc) as tc:
        _emit(nc, tc, xt_d, A_d, w_d, WvT_d, id_d, out_d)
    nc.compile()
    _CACHE[key] = nc
    return nc


def prepare_inputs(x, W, b):
    """Host-side prep: shard + transpose x over S, build derived matrices."""
    x = np.asarray(x, dtype=np.float32)
    W = np.asarray(W, dtype=np.float32)
    b = np.asarray(b, dtype=np.float32)

    rs = math.sqrt(float(D))
    Wq = W[0::3, :].astype(np.float64) / rs
    Wk = W[1::3, :].astype(np.float64)
    Wv = W[2::3, :]
    bq = b[0::3].astype(np.float64) / rs
    bv = b[2::3]

    A = (Wq.T @ Wk).astype(np.float16)                       # [128, 128]
    w = (Wk.T @ bq).astype(np.float32)[:, None]              # [128, 1]
    WvT = np.ascontiguousarray(Wv.T).astype(np.float16)      # [128, 128]
    ident = np.eye(D, dtype=np.float16)

    scale = _temporal_scale()                                # [1024]
    in_maps = []
    for c in range(NCORES):
        sl = slice(c * S_LOC, (c + 1) * S_LOC)
        xs_c = x[:, sl, :] * scale[sl][None, :, None]        # [n, s, d] f32
        xt_c = np.ascontiguousarray(
            xs_c.transpose(2, 1, 0)).reshape(D, COLS).astype(np.float16)
        in_maps.append({
            "xt": xt_c, "A": A, "w": w, "WvT": WvT, "ident": ident,
        })
    return in_maps, bv


def run(inputs, trace=False, **kw):
    nc = _build()
    in_maps, bv = prepare_inputs(inputs["x"], inputs["W"], inputs["b"])
    res = run_bass_kernel_spmd(nc, in_maps, core_ids=list(range(NCORES)),
                               trace=trace, **kw)
    out = np.concatenate(
        [res.results[c]["out"].astype(np.float32) for c in range(NCORES)], axis=1)
    out += np.tile(bv, S)[None, :]     # v-bias: attn rows sum to 1
    return out, res


def kernel(x, W, b):
    out, _ = run({"x": x, "W": W, "b": b})
    return out
